# revision 1
# baseline (speedup 1.0000x reference)
"""DynamicConv (MoE-routed 1x1 conv) Trainium2 kernel.

Data-parallel over batch: 8 cores x 4 samples. Each core:
  - routing MLP (3-layer, exact GELU) + softmax on its 4 samples
  - mixes the K=8 expert kernels per sample (DVE AXPY chain)
  - per-sample 256x256 @ 256x4096 matmul on TensorE (float32r single-pass)

Main matmuls run with float32r operands: fp32 bits streamed through the PE
in one pass (4x the fp32 matmul rate) at ~1.5e-4 relative error (vs 2.3e-3
for bf16). The kernel is then HBM-bound: ~36 MB/core at ~358 GB/s.

Problem constants are hardcoded (self-contained; no sibling imports):
  x [32, 256, 4096] f32, embedding [32, 128] f32,
  W0 [128,128], b0 [128], W1 [128,128], b1 [128], W2 [128,8], b2 [8],
  weight [8, 256, 256, 1], bias_k [8, 256]  -> out [32, 256, 4096] f32
"""

import numpy as np

import concourse.bacc as bacc
import concourse.mybir as mybir
import concourse.tile as tile
from concourse import bass_utils

F32 = mybir.dt.float32
F32R = mybir.dt.float32r
AF = mybir.ActivationFunctionType
ALU = mybir.AluOpType

MM_DT = F32R  # set to F32 for exact-fp32 main matmuls (4x slower PE)

N_CORES = 8
BS = 32
BPC = BS // N_CORES  # samples per core
IN_C = 256
OUT_C = 256
H = 4096
K = 8
D_EMBD = 128
HID = 128
N_IT = IN_C // 128   # input-channel tiles
N_OT = OUT_C // 128  # output-channel tiles
HC = 512             # h-chunk (one PSUM bank of fp32)
N_HC = H // HC

# params blob column layout [128 partitions x P_COLS]
C_W0 = 0
C_W1 = C_W0 + HID          # 128
C_W2 = C_W1 + HID          # 256
C_B0 = C_W2 + K            # 264
C_B1 = C_B0 + 1            # 265
C_EMBT = C_B1 + 1          # 266
C_B2 = C_EMBT + BPC        # 270  (partitions 0:K)
C_BK = C_B2 + 1            # 271  (partitions 0:K)
C_ID8 = C_BK + OUT_C       # 527  (partitions 0:K)
P_COLS = C_ID8 + K         # 535

_PROG = None  # compiled program cache


def _build_program(repeat=1):
    nc = bacc.Bacc("TRN2", target_bir_lowering=False, debug=False)

    xs = nc.dram_tensor("xs", [BPC, IN_C, H], MM_DT, kind="ExternalInput").ap()
    # wta[il, k, it, o] = weight[k, o, it*128+il]
    wta = nc.dram_tensor("wta", [128, K * N_IT * OUT_C], F32,
                         kind="ExternalInput").ap()
    params = nc.dram_tensor("params", [128, P_COLS], F32,
                            kind="ExternalInput").ap()
    out = nc.dram_tensor("out", [BPC, OUT_C, H], F32, kind="ExternalOutput").ap()

    with tile.TileContext(nc) as tc:
        with (
            tc.tile_pool(name="consts", bufs=1) as cpool,
            tc.tile_pool(name="rsb", bufs=1) as rsb,
            tc.tile_pool(name="rps", bufs=1, space="PSUM") as rps,
            tc.tile_pool(name="mix", bufs=4) as mixp,
            tc.tile_pool(name="xin", bufs=2) as xinp,
            tc.tile_pool(name="osb", bufs=4) as osbp,
            tc.tile_pool(name="mps", bufs=7, space="PSUM") as mps,
        ):
            # ---- constant loads (2 big DMAs, SWDGE so they overlap the
            # x/out HWDGE streams on their own queue) ----
            pa = cpool.tile([128, P_COLS], F32, tag="params")
            nc.gpsimd.dma_start(pa[:], params[:])
            w0_sb = pa[:, C_W0:C_W0 + HID]
            w1_sb = pa[:, C_W1:C_W1 + HID]
            w2_sb = pa[:, C_W2:C_W2 + K]
            b0_sb = pa[:, C_B0:C_B0 + 1]
            b1_sb = pa[:, C_B1:C_B1 + 1]
            embT_sb = pa[:, C_EMBT:C_EMBT + BPC]
            b2_sb = pa[0:K, C_B2:C_B2 + 1]
            bk_sb = pa[0:K, C_BK:C_BK + OUT_C]
            id8_sb = pa[0:K, C_ID8:C_ID8 + K]

            wtall = cpool.tile([128, K * N_IT * OUT_C], F32, tag="wtall")
            nc.gpsimd.dma_start(wtall[:], wta[:])

            def wt_sb(k, it):
                off = (k * N_IT + it) * OUT_C
                return wtall[:, off:off + OUT_C]

            ones_sb = cpool.tile([1, 128], F32, tag="ones")
            nc.vector.memset(ones_sb[:], 1.0)

            for rep in range(repeat):
                # ---- routing MLP (transposed orientation, all 4 samples) ----
                p1 = rps.tile([HID, BPC], F32, tag="rp")
                nc.tensor.matmul(p1[:], w0_sb, embT_sb, start=True, stop=True)
                h1_sb = rsb.tile([HID, BPC], F32, tag="h1")
                nc.scalar.activation(h1_sb[:], p1[:], AF.Gelu, bias=b0_sb)

                p2 = rps.tile([HID, BPC], F32, tag="rp")
                nc.tensor.matmul(p2[:], w1_sb, h1_sb[:], start=True, stop=True)
                h2_sb = rsb.tile([HID, BPC], F32, tag="h2")
                nc.scalar.activation(h2_sb[:], p2[:], AF.Gelu, bias=b1_sb)

                p3 = rps.tile([K, BPC], F32, tag="rp")
                nc.tensor.matmul(p3[:], w2_sb, h2_sb[:], start=True, stop=True)
                lT_sb = rsb.tile([K, BPC], F32, tag="lT")
                nc.scalar.activation(lT_sb[:], p3[:], AF.Identity, bias=b2_sb)

                # logitsT [K, BPC] -> logits [BPC, K]; softmax over free dim.
                # Logits are O(1) here so exp without max-subtraction is safe.
                p4 = rps.tile([BPC, K], F32, tag="rp")
                nc.tensor.transpose(p4[:], lT_sb[:], id8_sb)
                e_sb = rsb.tile([BPC, K], F32, tag="e")
                nc.scalar.activation(e_sb[:], p4[:], AF.Exp)
                s_sb = rsb.tile([BPC, 1], F32, tag="s")
                nc.vector.reduce_sum(s_sb[:], e_sb[:], axis=mybir.AxisListType.X)
                r_sb = rsb.tile([BPC, 1], F32, tag="r")
                nc.vector.reciprocal(r_sb[:], s_sb[:])
                att_sb = rsb.tile([BPC, K], F32, tag="att")
                nc.vector.tensor_scalar_mul(att_sb[:], e_sb[:], r_sb[:, 0:1])

                # attT [K, BPC] for the bias mix
                p5 = rps.tile([K, BPC], F32, tag="rp")
                nc.tensor.transpose(p5[:], att_sb[:], id8_sb[0:BPC, 0:BPC])
                attT_sb = rsb.tile([K, BPC], F32, tag="attT")
                nc.vector.tensor_copy(attT_sb[:], p5[:])

                # agg_bT[ot] [128, BPC] = bias_k[:, ot].T @ att.T
                aggb_sb = []
                for ot in range(N_OT):
                    p6 = rps.tile([128, BPC], F32, tag="rp")
                    nc.tensor.matmul(p6[:], bk_sb[:, ot * 128:(ot + 1) * 128],
                                     attT_sb[:], start=True, stop=True)
                    a = rsb.tile([128, BPC], F32, tag=f"aggb{ot}", name=f"aggb{ot}")
                    nc.vector.tensor_copy(a[:], p6[:])
                    aggb_sb.append(a)

                # broadcast att to all 128 partitions: attB [128, BPC*K]
                att_flat = rsb.tile([1, BPC * K], F32, tag="attf")
                nc.gpsimd.dma_start(att_flat[:], att_sb[:])
                p7 = rps.tile([128, BPC * K], F32, tag="rp")
                nc.tensor.matmul(p7[:], ones_sb[:], att_flat[:], start=True, stop=True)
                attB_sb = rsb.tile([128, BPC * K], F32, tag="attB")
                nc.vector.tensor_copy(attB_sb[:], p7[:])

                # ---- mix expert kernels + main per-sample matmul ----
                for b in range(BPC):
                    mixT = []
                    for it in range(N_IT):
                        m = mixp.tile([128, OUT_C], F32, tag=f"mix{it}",
                                      name=f"mix_b{b}_{it}")
                        a0 = attB_sb[:, b * K:b * K + 1]
                        nc.vector.tensor_scalar_mul(m[:], wt_sb(0, it), a0)
                        for k in range(1, K - 1):
                            ak = attB_sb[:, b * K + k:b * K + k + 1]
                            nc.vector.scalar_tensor_tensor(
                                m[:], wt_sb(k, it), ak, m[:],
                                op0=ALU.mult, op1=ALU.add)
                        # last AXPY rounds the accumulator into f32r
                        mr = mixp.tile([128, OUT_C], MM_DT, tag=f"mixr{it}",
                                       name=f"mixr_b{b}_{it}")
                        ak = attB_sb[:, b * K + (K - 1):b * K + K]
                        nc.vector.scalar_tensor_tensor(
                            mr[:], wt_sb(K - 1, it), ak, m[:],
                            op0=ALU.mult, op1=ALU.add)
                        mixT.append(mr)

                    # two 2 MB DMAs per sample (always on SP's ring: reads
                    # depend only on slot release, never behind compute).
                    # Separate tiles let the it=0 matmuls start while the
                    # it=1 half is still in flight — shortens the tail.
                    x_t = []
                    for it in range(N_IT):
                        xh = xinp.tile([128, H], MM_DT, tag=f"x{it}",
                                       name=f"x_b{b}_{it}")
                        nc.sync.dma_start(xh[:],
                                          xs[b, it * 128:(it + 1) * 128, :])
                        x_t.append(xh)

                    for ot in range(N_OT):
                        o_sb = osbp.tile([128, H], F32, tag="o",
                                         name=f"o_b{b}_{ot}")
                        for hc in range(N_HC):
                            ps = mps.tile([128, HC], F32, tag="mm")
                            for it in range(N_IT):
                                nc.tensor.matmul(
                                    ps[:],
                                    mixT[it][:, ot * 128:(ot + 1) * 128],
                                    x_t[it][:, hc * HC:(hc + 1) * HC],
                                    start=(it == 0), stop=(it == N_IT - 1))
                            dst = o_sb[:, hc * HC:(hc + 1) * HC]
                            bias_ap = aggb_sb[ot][:, b:b + 1]
                            if hc % 2 == 0:
                                nc.scalar.activation(dst, ps[:], AF.Identity,
                                                     bias=bias_ap)
                            else:
                                nc.vector.tensor_scalar(dst, ps[:], bias_ap, None,
                                                        op0=ALU.add)
                        # each output tile leaves as two 1 MB halves on the
                        # two DMA paths that don't carry the x reads (ACT
                        # HWDGE + GpSimd SWDGE): writes overlap reads, SP's
                        # read stream stays uncontended, tail transfer halves
                        orows = out[b, ot * 128:(ot + 1) * 128, :]
                        nc.gpsimd.dma_start(orows[:, 0:H // 2],
                                            o_sb[:, 0:H // 2])
                        nc.scalar.dma_start(orows[:, H // 2:H],
                                            o_sb[:, H // 2:H])

    nc.compile()
    return nc


def _get_program():
    global _PROG
    if _PROG is None:
        _PROG = _build_program()
    return _PROG


def build_in_maps(inputs):
    x = np.ascontiguousarray(np.asarray(inputs["x"], dtype=np.float32))
    emb = np.asarray(inputs["embedding"], dtype=np.float32)
    W0 = np.asarray(inputs["W0"], dtype=np.float32)
    b0 = np.asarray(inputs["b0"], dtype=np.float32)
    W1 = np.asarray(inputs["W1"], dtype=np.float32)
    b1 = np.asarray(inputs["b1"], dtype=np.float32)
    W2 = np.asarray(inputs["W2"], dtype=np.float32)
    b2 = np.asarray(inputs["b2"], dtype=np.float32)
    weight = np.asarray(inputs["weight"], dtype=np.float32)[..., 0]  # [K, O, I]
    bias_k = np.asarray(inputs["bias_k"], dtype=np.float32)

    # wta[il, (k, it, o)] = weight[k, o, it*128+il]
    wta = np.ascontiguousarray(
        weight.transpose(2, 0, 1)           # [I, K, O]
        .reshape(N_IT, 128, K, OUT_C)       # [it, il, K, O]
        .transpose(1, 2, 0, 3)              # [il, K, it, O]
        .reshape(128, K * N_IT * OUT_C))

    base = np.zeros((128, P_COLS), dtype=np.float32)
    base[:, C_W0:C_W0 + HID] = W0
    base[:, C_W1:C_W1 + HID] = W1
    base[:, C_W2:C_W2 + K] = W2
    base[:, C_B0] = b0
    base[:, C_B1] = b1
    base[0:K, C_B2] = b2
    base[0:K, C_BK:C_BK + OUT_C] = bias_k
    base[0:K, C_ID8:C_ID8 + K] = np.eye(K, dtype=np.float32)

    in_maps = []
    for c in range(N_CORES):
        sl = slice(c * BPC, (c + 1) * BPC)
        p = base.copy()
        p[:, C_EMBT:C_EMBT + BPC] = emb[sl].T
        in_maps.append({
            "xs": np.ascontiguousarray(x[sl]),
            "wta": wta,
            "params": p,
        })
    return in_maps


def run(inputs, trace=False):
    nc = _get_program()
    in_maps = build_in_maps(inputs)
    res = bass_utils.run_bass_kernel_spmd(
        nc, in_maps, core_ids=list(range(N_CORES)), trace=trace)
    out = np.concatenate([res.results[c]["out"] for c in range(N_CORES)], axis=0)
    return out, res


def kernel(**inputs):
    out, _ = run(inputs, trace=False)
    return out

